# revision 21
# baseline (speedup 1.0000x reference)
"""Trainium2 Bass kernel for EuclideanCodebook (VQ) forward.

Problem: x [16, 4096, 256] f32, embed [2048, 256] f32.
  dist[t, k] = -(||x_t||^2 - 2 x_t.e_k + ||e_k||^2)
  ind[t]     = argmax_k dist  (== argmax_k 2 x_t.e_k - ||e_k||^2)
  quantize   = embed[ind]

Sharding: data-parallel over the flattened token axis (65536 tokens ->
8192/core on 8 cores), codebook replicated.

Per-core pipeline (128-token tiles, 64 tiles):
  PE  : score = x @ (2 embed).T via an exact fp16 two-term split
        (x_hi*e_hi + x_hi*e_lo + x_lo*e_hi, fp32 PSUM accumulation;
        max |error| vs fp64 is ~1.5e-5 while the smallest argmax gap of
        the workload distribution at this size is ~3e-5; plain fp32
        matmul is 4x slower per row on the PE, the fp32r path is TF32
        and numerically unusable). For `alpha` of the four 512-wide
        k-chunks the -||e||^2 bias is accumulated by two fp16 rank-1
        matmul rows (ones.T @ [b_hi; b_lo]); the remaining chunks get
        the bias via a DVE in-place PSUM add, balancing PE vs DVE time.
  DVE : tensor_tensor add (bias, in-place in PSUM), then max (top-8) +
        max_index (first-occurrence argmax, exact jnp tie semantics).
  DMA : gpsimd single-offset indirect gather embed[idx] -> SBUF, HWDGE
        write to DRAM. (Multi-offset indirect gathers return garbage on
        this HW; tensor_tensor_reduce crashes the device.)

Host side: x is pre-transposed per core ([256, 8192]) and pre-split into
fp16 hi/lo parts so the contraction dim lands on SBUF partitions without
on-device transposes or converts.
"""

import numpy as np

import concourse.bass as bass
import concourse.mybir as mybir
import concourse.tile as tile
from concourse import bacc
from concourse.bass_utils import run_bass_kernel_spmd

P = 128
D = 256
K = 2048
KC = 512  # k chunk (one PSUM bank of fp32)
NCORES = 8

F32 = mybir.dt.float32
F16 = mybir.dt.float16
I32 = mybir.dt.int32


def build_nc(ntok: int, group: int = 8, alpha: int = 1, mode: str = "fp16",
             repeat: int = 1):
    """Per-core Bass program for `ntok` tokens. `alpha` in [0,4]: number of
    k-chunks whose bias rides the PE as fp16 rank-1 rows (rest on DVE).
    mode="fp32" uses plain fp32 matmuls (alpha forced to 0). `repeat`
    replicates the whole body (for on-device timing of one body)."""
    if mode == "fp32":
        alpha = 0
    nt = ntok // P
    assert nt % group == 0
    ng = nt // group
    nkc = K // KC

    nc = bacc.Bacc("TRN2", target_bir_lowering=False, debug=False,
                   num_devices=NCORES)

    XDT = F16 if mode == "fp16" else F32
    xhi_d = nc.dram_tensor("x_hi", [D, ntok], XDT, kind="ExternalInput").ap()
    xlo_d = nc.dram_tensor("x_lo", [D, ntok], F16, kind="ExternalInput").ap()
    ehi_d = nc.dram_tensor("e_hi", [D, K], XDT, kind="ExternalInput").ap()
    elo_d = nc.dram_tensor("e_lo", [D, K], F16, kind="ExternalInput").ap()
    bhi_d = nc.dram_tensor("b_hi", [1, K], F16, kind="ExternalInput").ap()
    blo_d = nc.dram_tensor("b_lo", [1, K], F16, kind="ExternalInput").ap()
    ones_d = nc.dram_tensor("ones", [1, P], F16, kind="ExternalInput").ap()
    negb_d = nc.dram_tensor("negb", [P, K], F32, kind="ExternalInput").ap()
    embed_d = nc.dram_tensor("embed", [K, D], F32, kind="ExternalInput").ap()

    q_d = nc.dram_tensor("quantize", [ntok, D], F32, kind="ExternalOutput").ap()
    ind_d = nc.dram_tensor("eind", [nt, P], I32, kind="ExternalOutput").ap()

    from contextlib import ExitStack
    with tile.TileContext(nc) as tc, ExitStack() as ctx:
        const_pool = ctx.enter_context(tc.tile_pool(name="const", bufs=1))
        xg_pool = ctx.enter_context(tc.tile_pool(name="xg", bufs=3))
        psum_pool = ctx.enter_context(tc.tile_pool(name="psum", bufs=2,
                                                   space="PSUM"))
        dist_pool = ctx.enter_context(tc.tile_pool(name="dist", bufs=3))
        m_pool = ctx.enter_context(tc.tile_pool(name="m", bufs=3))
        q_pool = ctx.enter_context(tc.tile_pool(name="q", bufs=3))

        # one-time constant loads
        ehi_sb, elo_sb = [], []
        for d in range(2):
            t = const_pool.tile([P, K], XDT, tag=f"ehi{d}")
            nc.sync.dma_start(t[:], ehi_d[d * P:(d + 1) * P, :])
            ehi_sb.append(t)
            if mode == "fp16":
                t = const_pool.tile([P, K], F16, tag=f"elo{d}")
                nc.sync.dma_start(t[:], elo_d[d * P:(d + 1) * P, :])
                elo_sb.append(t)
        bhi_sb = const_pool.tile([1, K], F16, tag="bhi")
        nc.sync.dma_start(bhi_sb[:], bhi_d[:])
        blo_sb = const_pool.tile([1, K], F16, tag="blo")
        nc.sync.dma_start(blo_sb[:], blo_d[:])
        ones_sb = const_pool.tile([1, P], F16, tag="ones")
        nc.sync.dma_start(ones_sb[:], ones_d[:])
        if alpha < nkc:
            negb_sb = const_pool.tile([P, K], F32, tag="negb")
            nc.sync.dma_start(negb_sb[:], negb_d[:])
        idx_all = const_pool.tile([P, nt], I32, tag="idxall")

        for _rep in range(repeat):
          for g in range(ng):
            gsl = slice(g * group * P, (g + 1) * group * P)
            xhi, xlo = [], []
            for d in range(2):
                t = xg_pool.tile([P, group * P], XDT, tag=f"xhi{d}")
                nc.sync.dma_start(t[:], xhi_d[d * P:(d + 1) * P, gsl])
                xhi.append(t)
                if mode == "fp16":
                    t = xg_pool.tile([P, group * P], F16, tag=f"xlo{d}")
                    nc.sync.dma_start(t[:], xlo_d[d * P:(d + 1) * P, gsl])
                    xlo.append(t)
            for i in range(group):
                t = g * group + i
                isl = slice(i * P, (i + 1) * P)
                ps = psum_pool.tile([P, K], F32, tag="ps")
                dist = dist_pool.tile([P, K], F32, tag="dist")
                for kc in range(nkc):
                    sl = slice(kc * KC, (kc + 1) * KC)
                    first = True
                    if kc < alpha:
                        nc.tensor.matmul(ps[:, sl], lhsT=ones_sb[:],
                                         rhs=bhi_sb[:, sl],
                                         start=True, stop=False)
                        nc.tensor.matmul(ps[:, sl], lhsT=ones_sb[:],
                                         rhs=blo_sb[:, sl],
                                         start=False, stop=False)
                        first = False
                    if mode == "fp16":
                        prods = [(xhi[0], ehi_sb[0]), (xhi[1], ehi_sb[1]),
                                 (xhi[0], elo_sb[0]), (xhi[1], elo_sb[1]),
                                 (xlo[0], ehi_sb[0]), (xlo[1], ehi_sb[1])]
                    else:
                        prods = [(xhi[0], ehi_sb[0]), (xhi[1], ehi_sb[1])]
                    for j, (xt, et) in enumerate(prods):
                        nc.tensor.matmul(ps[:, sl], lhsT=xt[:, isl],
                                         rhs=et[:, sl],
                                         start=first and j == 0,
                                         stop=j == len(prods) - 1)
                    if kc < alpha:
                        # bias already accumulated by the PE rank-1 rows;
                        # stage to SBUF on the otherwise idle ACT engine
                        nc.scalar.copy(dist[:, sl], ps[:, sl])
                    elif kc == nkc - 1:
                        # bias for all DVE-owned chunks in one instruction
                        msl = slice(alpha * KC, K)
                        nc.vector.tensor_tensor(
                            out=dist[:, msl], in0=ps[:, msl],
                            in1=negb_sb[:, msl], op=mybir.AluOpType.add)
                # top-8 + first-occurrence argmax
                m8 = m_pool.tile([P, 8], F32, tag="m8")
                nc.vector.max(out=m8[:], in_=dist[:])
                i8 = m_pool.tile([P, 8], mybir.dt.uint32, tag="i8")
                nc.vector.max_index(out=i8[:], in_max=m8[:], in_values=dist[:])
                nc.vector.tensor_copy(idx_all[:, t:t + 1],
                                      i8[:, 0:1].bitcast(I32))

                # gather embed rows -> quantize
                qt = q_pool.tile([P, D], F32, tag="qt")
                nc.gpsimd.indirect_dma_start(
                    out=qt[:],
                    out_offset=None,
                    in_=embed_d[:],
                    in_offset=bass.IndirectOffsetOnAxis(
                        ap=idx_all[:, t:t + 1], axis=0),
                )
                nc.sync.dma_start(q_d[t * P:(t + 1) * P, :], qt[:])

        nc.sync.dma_start(ind_d.rearrange("t p -> p t"), idx_all[:])

    nc.compile()
    return nc


def make_host_inputs(x_core: np.ndarray, embed: np.ndarray,
                     mode: str = "fp16"):
    """Per-core input map. x_core [ntok, D] f32, embed [K, D] f32."""
    xt = np.ascontiguousarray(x_core.T)  # [D, ntok] f32
    e2 = np.ascontiguousarray((embed.astype(np.float32) * np.float32(2.0)).T)
    if mode == "fp16":
        x_hi = xt.astype(np.float16)
        x_lo = (xt - x_hi.astype(np.float32)).astype(np.float16)
        e_hi = e2.astype(np.float16)
        e_lo = (e2 - e_hi.astype(np.float32)).astype(np.float16)
    else:
        x_hi = xt
        x_lo = np.zeros_like(xt, dtype=np.float16)
        e_hi = e2
        e_lo = np.zeros_like(e2, dtype=np.float16)
    bneg = (-(embed.astype(np.float64) ** 2).sum(1)).astype(np.float32)
    b_hi = bneg.astype(np.float16)
    b_lo = (bneg - b_hi.astype(np.float32)).astype(np.float16)
    return {
        "x_hi": x_hi,
        "x_lo": x_lo,
        "e_hi": e_hi,
        "e_lo": e_lo,
        "b_hi": np.ascontiguousarray(b_hi.reshape(1, K)),
        "b_lo": np.ascontiguousarray(b_lo.reshape(1, K)),
        "ones": np.ones((1, P), np.float16),
        "negb": np.ascontiguousarray(np.broadcast_to(bneg, (P, K))),
        "embed": np.ascontiguousarray(embed.astype(np.float32)),
    }


_CACHE = {}


def _get_nc(ntok):
    if ntok not in _CACHE:
        _CACHE[ntok] = build_nc(ntok)
    return _CACHE[ntok]


def kernel(x: np.ndarray, embed: np.ndarray):
    x = np.asarray(x, dtype=np.float32)
    embed = np.asarray(embed, dtype=np.float32)
    shape = x.shape
    xf = x.reshape(-1, shape[-1])
    n = xf.shape[0]
    assert n % NCORES == 0
    npc = n // NCORES

    nc = _get_nc(npc)
    in_maps = [make_host_inputs(xf[c * npc:(c + 1) * npc], embed)
               for c in range(NCORES)]
    res = run_bass_kernel_spmd(nc, in_maps, list(range(NCORES))).results

    q = np.concatenate([res[c]["quantize"] for c in range(NCORES)], axis=0)
    ind = np.concatenate([res[c]["eind"].reshape(-1) for c in range(NCORES)])
    quantize = q.reshape(*shape)
    embed_ind = ind.astype(np.int32).reshape(*shape[:-1])
    return quantize, embed_ind


# revision 22
# speedup vs baseline: 1.8871x; 1.8871x over previous
"""Trainium2 Bass kernel for EuclideanCodebook (VQ) forward.

Problem: x [16, 4096, 256] f32, embed [2048, 256] f32.
  dist[t, k] = -(||x_t||^2 - 2 x_t.e_k + ||e_k||^2)
  ind[t]     = argmax_k dist  (== argmax_k 2 x_t.e_k - ||e_k||^2)
  quantize   = embed[ind]

Sharding: data-parallel over the flattened token axis (65536 tokens ->
8192/core on 8 cores), codebook replicated.

Per-core pipeline (128-token tiles, 64 tiles):
  PE  : score = x @ (2 embed).T via an exact fp16 two-term split
        (x_hi*e_hi + x_hi*e_lo + x_lo*e_hi, fp32 PSUM accumulation;
        max |error| vs fp64 is ~1.5e-5 while the smallest argmax gap of
        the workload distribution at this size is ~3e-5; plain fp32
        matmul is 4x slower per row on the PE, the fp32r path is TF32
        and numerically unusable). For `alpha` of the four 512-wide
        k-chunks the -||e||^2 bias is accumulated by two fp16 rank-1
        matmul rows (ones.T @ [b_hi; b_lo]); the remaining chunks get
        the bias via a DVE in-place PSUM add, balancing PE vs DVE time.
  DVE : tensor_tensor add (bias, in-place in PSUM), then max (top-8) +
        max_index (first-occurrence argmax, exact jnp tie semantics).
  DMA : gpsimd single-offset indirect gather embed[idx] -> SBUF, HWDGE
        write to DRAM. (Multi-offset indirect gathers return garbage on
        this HW; tensor_tensor_reduce crashes the device.)

Host side: x is pre-transposed per core ([256, 8192]) and pre-split into
fp16 hi/lo parts so the contraction dim lands on SBUF partitions without
on-device transposes or converts.
"""

import numpy as np

import concourse.bass as bass
import concourse.mybir as mybir
import concourse.tile as tile
from concourse import bacc
from concourse.bass_utils import run_bass_kernel_spmd

P = 128
D = 256
K = 2048
KC = 512  # k chunk (one PSUM bank of fp32)
NCORES = 8

F32 = mybir.dt.float32
F16 = mybir.dt.float16
I32 = mybir.dt.int32


def build_nc(ntok: int, group: int = 8, alpha: int = 1, mode: str = "fp16",
             repeat: int = 1):
    """Per-core Bass program for `ntok` tokens. `alpha` in [0,4]: number of
    k-chunks whose bias rides the PE as fp16 rank-1 rows (rest on DVE).
    mode="fp32" uses plain fp32 matmuls (alpha forced to 0). `repeat`
    replicates the whole body (for on-device timing of one body)."""
    if mode == "fp32":
        alpha = 0
    nt = ntok // P
    assert nt % group == 0
    ng = nt // group
    nkc = K // KC

    nc = bacc.Bacc("TRN2", target_bir_lowering=False, debug=False,
                   num_devices=NCORES)

    XDT = F16 if mode == "fp16" else F32
    xhi_d = nc.dram_tensor("x_hi", [D, ntok], XDT, kind="ExternalInput").ap()
    xlo_d = nc.dram_tensor("x_lo", [D, ntok], F16, kind="ExternalInput").ap()
    ehi_d = nc.dram_tensor("e_hi", [D, K], XDT, kind="ExternalInput").ap()
    elo_d = nc.dram_tensor("e_lo", [D, K], F16, kind="ExternalInput").ap()
    b2_d = nc.dram_tensor("b2", [2, K], F16, kind="ExternalInput").ap()
    ones_d = nc.dram_tensor("ones", [2, P], F16, kind="ExternalInput").ap()
    negb_d = nc.dram_tensor("negb", [P, K], F32, kind="ExternalInput").ap()
    embed_d = nc.dram_tensor("embed", [K, D], F32, kind="ExternalInput").ap()

    q_d = nc.dram_tensor("quantize", [ntok, D], F32, kind="ExternalOutput").ap()
    ind_d = nc.dram_tensor("eind", [nt, P], I32, kind="ExternalOutput").ap()

    from contextlib import ExitStack
    with tile.TileContext(nc) as tc, ExitStack() as ctx:
        const_pool = ctx.enter_context(tc.tile_pool(name="const", bufs=1))
        xg_pool = ctx.enter_context(tc.tile_pool(name="xg", bufs=3))
        psum_pool = ctx.enter_context(tc.tile_pool(name="psum", bufs=2,
                                                   space="PSUM"))
        dist_pool = ctx.enter_context(tc.tile_pool(name="dist", bufs=3))
        m_pool = ctx.enter_context(tc.tile_pool(name="m", bufs=3))
        q_pool = ctx.enter_context(tc.tile_pool(name="q", bufs=3))

        # one-time constant loads
        ehi_sb, elo_sb = [], []
        for d in range(2):
            t = const_pool.tile([P, K], XDT, tag=f"ehi{d}")
            nc.sync.dma_start(t[:], ehi_d[d * P:(d + 1) * P, :])
            ehi_sb.append(t)
            if mode == "fp16":
                t = const_pool.tile([P, K], F16, tag=f"elo{d}")
                nc.sync.dma_start(t[:], elo_d[d * P:(d + 1) * P, :])
                elo_sb.append(t)
        b2_sb = const_pool.tile([2, K], F16, tag="b2")
        nc.sync.dma_start(b2_sb[:], b2_d[:])
        ones_sb = const_pool.tile([2, P], F16, tag="ones")
        nc.sync.dma_start(ones_sb[:], ones_d[:])
        if alpha < nkc:
            negb_sb = const_pool.tile([P, K], F32, tag="negb")
            nc.sync.dma_start(negb_sb[:], negb_d[:])
        idx_all = const_pool.tile([P, nt], I32, tag="idxall")

        for _rep in range(repeat):
          for g in range(ng):
            gsl = slice(g * group * P, (g + 1) * group * P)
            xhi, xlo = [], []
            for d in range(2):
                t = xg_pool.tile([P, group * P], XDT, tag=f"xhi{d}")
                nc.sync.dma_start(t[:], xhi_d[d * P:(d + 1) * P, gsl])
                xhi.append(t)
                if mode == "fp16":
                    t = xg_pool.tile([P, group * P], F16, tag=f"xlo{d}")
                    nc.sync.dma_start(t[:], xlo_d[d * P:(d + 1) * P, gsl])
                    xlo.append(t)
            for i in range(group):
                t = g * group + i
                isl = slice(i * P, (i + 1) * P)
                ps = psum_pool.tile([P, K], F32, tag="ps")
                dist = dist_pool.tile([P, K], F32, tag="dist")
                for kc in range(nkc):
                    sl = slice(kc * KC, (kc + 1) * KC)
                    first = True
                    if kc < alpha:
                        # one matmul adds b_hi + b_lo (contraction dim 2)
                        nc.tensor.matmul(ps[:, sl], lhsT=ones_sb[:],
                                         rhs=b2_sb[:, sl],
                                         start=True, stop=False)
                        first = False
                    if mode == "fp16":
                        prods = [(xhi[0], ehi_sb[0]), (xhi[1], ehi_sb[1]),
                                 (xhi[0], elo_sb[0]), (xhi[1], elo_sb[1]),
                                 (xlo[0], ehi_sb[0]), (xlo[1], ehi_sb[1])]
                    else:
                        prods = [(xhi[0], ehi_sb[0]), (xhi[1], ehi_sb[1])]
                    for j, (xt, et) in enumerate(prods):
                        nc.tensor.matmul(ps[:, sl], lhsT=xt[:, isl],
                                         rhs=et[:, sl],
                                         start=first and j == 0,
                                         stop=j == len(prods) - 1)
                    if kc < alpha:
                        # bias already accumulated by the PE rank-1 rows;
                        # stage to SBUF on the otherwise idle ACT engine
                        nc.scalar.copy(dist[:, sl], ps[:, sl])
                    elif kc == nkc - 1:
                        # bias for all DVE-owned chunks in one instruction
                        msl = slice(alpha * KC, K)
                        nc.vector.tensor_tensor(
                            out=dist[:, msl], in0=ps[:, msl],
                            in1=negb_sb[:, msl], op=mybir.AluOpType.add)
                # top-8 + first-occurrence argmax
                m8 = m_pool.tile([P, 8], F32, tag="m8")
                nc.vector.max(out=m8[:], in_=dist[:])
                i8 = m_pool.tile([P, 8], mybir.dt.uint32, tag="i8")
                nc.vector.max_index(out=i8[:], in_max=m8[:], in_values=dist[:])
                nc.vector.tensor_copy(idx_all[:, t:t + 1],
                                      i8[:, 0:1].bitcast(I32))

                # gather embed rows -> quantize
                qt = q_pool.tile([P, D], F32, tag="qt")
                nc.gpsimd.indirect_dma_start(
                    out=qt[:],
                    out_offset=None,
                    in_=embed_d[:],
                    in_offset=bass.IndirectOffsetOnAxis(
                        ap=idx_all[:, t:t + 1], axis=0),
                )
                nc.sync.dma_start(q_d[t * P:(t + 1) * P, :], qt[:])

        nc.sync.dma_start(ind_d.rearrange("t p -> p t"), idx_all[:])

    nc.compile()
    return nc


def make_host_inputs(x_core: np.ndarray, embed: np.ndarray,
                     mode: str = "fp16"):
    """Per-core input map. x_core [ntok, D] f32, embed [K, D] f32."""
    xt = np.ascontiguousarray(x_core.T)  # [D, ntok] f32
    e2 = np.ascontiguousarray((embed.astype(np.float32) * np.float32(2.0)).T)
    if mode == "fp16":
        x_hi = xt.astype(np.float16)
        x_lo = (xt - x_hi.astype(np.float32)).astype(np.float16)
        e_hi = e2.astype(np.float16)
        e_lo = (e2 - e_hi.astype(np.float32)).astype(np.float16)
    else:
        x_hi = xt
        x_lo = np.zeros_like(xt, dtype=np.float16)
        e_hi = e2
        e_lo = np.zeros_like(e2, dtype=np.float16)
    bneg = (-(embed.astype(np.float64) ** 2).sum(1)).astype(np.float32)
    b_hi = bneg.astype(np.float16)
    b_lo = (bneg - b_hi.astype(np.float32)).astype(np.float16)
    return {
        "x_hi": x_hi,
        "x_lo": x_lo,
        "e_hi": e_hi,
        "e_lo": e_lo,
        "b2": np.ascontiguousarray(np.stack([b_hi, b_lo])),
        "ones": np.ones((2, P), np.float16),
        "negb": np.ascontiguousarray(np.broadcast_to(bneg, (P, K))),
        "embed": np.ascontiguousarray(embed.astype(np.float32)),
    }


_CACHE = {}


def _get_nc(ntok):
    if ntok not in _CACHE:
        _CACHE[ntok] = build_nc(ntok)
    return _CACHE[ntok]


def kernel(x: np.ndarray, embed: np.ndarray):
    x = np.asarray(x, dtype=np.float32)
    embed = np.asarray(embed, dtype=np.float32)
    shape = x.shape
    xf = x.reshape(-1, shape[-1])
    n = xf.shape[0]
    assert n % NCORES == 0
    npc = n // NCORES

    nc = _get_nc(npc)
    in_maps = [make_host_inputs(xf[c * npc:(c + 1) * npc], embed)
               for c in range(NCORES)]
    res = run_bass_kernel_spmd(nc, in_maps, list(range(NCORES))).results

    q = np.concatenate([res[c]["quantize"] for c in range(NCORES)], axis=0)
    ind = np.concatenate([res[c]["eind"].reshape(-1) for c in range(NCORES)])
    quantize = q.reshape(*shape)
    embed_ind = ind.astype(np.int32).reshape(*shape[:-1])
    return quantize, embed_ind
